# revision 1
# baseline (speedup 1.0000x reference)
"""HTM spatial-pooler kernel for Trainium2 (8 NeuronCores, data-parallel over tokens).

Computes, for x = input_vector reshaped to [4096 tokens, 4096]:
    overlap = x @ C^T               (C = connections [2048, 4096], binary)
    boosted = overlap * boost       (per-column boosting factors)
    masked  = where(boosted >= kth_largest_per_row(boosted, k), boosted, 0)

Strategy per core (512 tokens):
  - Matmul as two bf16 passes (x = x_hi + x_lo split host-side; C is exactly
    representable in bf16) accumulated in fp32 PSUM -> fp32-level accuracy at
    bf16 PE throughput. C^T stays resident in SBUF (16 MB bf16).
  - Tokens on PSUM partitions, columns on the free axis, so the per-row top-k
    runs on the DVE with max8/match_replace; the k-th value is used as a
    threshold and the mask applied with tensor_scalar(is_ge) + multiply
    (matches the reference's `boosted >= threshold` tie semantics).
"""
import math

import numpy as np
import ml_dtypes

import concourse.bacc as bacc
import concourse.mybir as mybir
from concourse import tile
from concourse.bass_utils import run_bass_kernel_spmd

BF16 = mybir.dt.bfloat16
F32 = mybir.dt.float32

N_CORES = 8
TOK_PER_CORE = 512
M_TILES = 4          # 128-token tiles per core
D = 4096             # input size (contraction)
KC = D // 128        # 32 contraction chunks
NCOL = 2048          # minicolumns
NCH = NCOL // 512    # 4 psum column chunks

_BUILD_CACHE = {}


def _build(k_active: int):
    nc = bacc.Bacc("TRN2", target_bir_lowering=False)
    xhi = nc.dram_tensor("xhi", [M_TILES, 128, KC * 128], BF16, kind="ExternalInput")
    xlo = nc.dram_tensor("xlo", [M_TILES, 128, KC * 128], BF16, kind="ExternalInput")
    ct = nc.dram_tensor("ct", [128, KC * NCOL], BF16, kind="ExternalInput")
    bc = nc.dram_tensor("bc", [128, NCOL], F32, kind="ExternalInput")
    out = nc.dram_tensor("out", [M_TILES, 128, NCOL], F32, kind="ExternalOutput")

    rounds = max(1, math.ceil(k_active / 8))
    t_idx = (k_active - 1) % 8

    with tile.TileContext(nc) as tc:
        with (
            tc.tile_pool(name="cpool", bufs=1) as cpool,
            tc.tile_pool(name="xpool", bufs=2) as xpool,
            tc.tile_pool(name="psum", bufs=2, space="PSUM") as pspool,
            tc.tile_pool(name="work", bufs=1) as wpool,
            tc.tile_pool(name="bpool", bufs=2) as bpool,
            tc.tile_pool(name="lpool", bufs=1) as lpool,
        ):
            XCH = 4                      # x loaded in 4 kc-block chunks
            KCB = KC // XCH              # 8 kc per chunk

            def load_x(m):
                chunks = []
                for j in range(XCH):
                    xhj = xpool.tile([128, KCB * 128], BF16, tag=f"xh{j}")
                    xlj = xpool.tile([128, KCB * 128], BF16, tag=f"xl{j}")
                    nc.sync.dma_start(
                        xhj[:], xhi[m][:, j * KCB * 128:(j + 1) * KCB * 128])
                    nc.sync.dma_start(
                        xlj[:], xlo[m][:, j * KCB * 128:(j + 1) * KCB * 128])
                    chunks.append((xhj, xlj))
                return chunks

            # C^T resident as per-kc chunk tiles so the first matmuls only
            # gate on the first chunk's DMA, not the full 16 MB load. The
            # first two chunks are issued before m=0's x prefetch (and the
            # rest after it) so neither first-matmul operand queues behind
            # the other's bulk traffic.
            ct_tiles = []

            def load_ct(kc):
                t = cpool.tile([128, NCOL], BF16, tag=f"ct{kc}")
                nc.sync.dma_start(t[:], ct[:, kc * NCOL:(kc + 1) * NCOL])
                ct_tiles.append(t)

            load_ct(0)
            load_ct(1)
            next_xchunks = load_x(0)
            for kc in range(2, KC):
                load_ct(kc)
            bc_t = cpool.tile([128, NCOL], F32)
            nc.sync.dma_start(bc_t[:], bc[:])

            for m in range(M_TILES):
                xchunks = next_xchunks
                if m + 1 < M_TILES:
                    next_xchunks = load_x(m + 1)

                ps = pspool.tile([128, NCOL], F32)
                for kc in range(KC):
                    pair = xchunks[kc // KCB]
                    off = (kc % KCB) * 128
                    for si in (0, 1):
                        lhsT = pair[si][:, off:off + 128]
                        for n in range(NCH):
                            nc.tensor.matmul(
                                ps[:, n * 512:(n + 1) * 512],
                                lhsT,
                                ct_tiles[kc][:, n * 512:(n + 1) * 512],
                                start=(kc == 0 and si == 0),
                                stop=(kc == KC - 1 and si == 1),
                            )

                boosted = bpool.tile([128, NCOL], F32, tag="boosted")
                nc.vector.tensor_tensor(
                    boosted[:], ps[:], bc_t[:], mybir.AluOpType.mult
                )

                if k_active <= 48:
                    # Segmented top-k: per-64-col-segment top-8 candidates
                    # (a segment can contribute at most 8 to the top-k; for
                    # k=40 the chance any segment holds >8 of the top-k is
                    # ~2e-4 per row), then an exact k-th-largest on the 256
                    # candidates, then threshold-mask the full row (same
                    # `>= thr` tie semantics as the reference).
                    SEG = 64
                    NSEG = NCOL // SEG
                    cands = wpool.tile([128, NSEG * 8], F32, tag="cands")
                    for s in range(NSEG):
                        nc.vector.max(
                            cands[:, s * 8:(s + 1) * 8],
                            boosted[:, s * SEG:(s + 1) * SEG],
                        )
                    tops = wpool.tile([128, 8 * rounds], F32, tag="tops")
                    wc = wpool.tile([128, NSEG * 8], F32, tag="wc")
                    src = cands
                    for r in range(rounds):
                        m8 = tops[:, r * 8:(r + 1) * 8]
                        nc.vector.max(m8, src[:])
                        if r != rounds - 1:
                            nc.vector.match_replace(wc[:], m8, src[:], 0.0)
                            src = wc
                    thr = tops[:, (rounds - 1) * 8 + t_idx:
                               (rounds - 1) * 8 + t_idx + 1]
                    mask = lpool.tile([128, NCOL], F32, tag="mask")
                    nc.vector.tensor_scalar(
                        mask[:], boosted[:], thr, None, mybir.AluOpType.is_ge
                    )
                    nc.vector.tensor_tensor(
                        mask[:], boosted[:], mask[:], mybir.AluOpType.mult
                    )
                    nc.sync.dma_start(out[m], mask[:])
                else:
                    # Exact full-width chain: zero the top-k in a working
                    # copy, then masked = boosted - working.
                    rem = k_active % 8
                    tops = wpool.tile([128, 8 * rounds], F32, tag="tops")
                    w = wpool.tile([128, NCOL], F32, tag="w")
                    src = boosted
                    for r in range(rounds):
                        m8 = tops[:, r * 8:(r + 1) * 8]
                        nc.vector.max(m8, src[:])
                        if r == rounds - 1 and rem:
                            nc.gpsimd.memset(m8[:, rem:], -1e30)
                        nc.vector.match_replace(w[:], m8, src[:], 0.0)
                        src = w
                    losers = lpool.tile([128, NCOL], F32, tag="losers")
                    nc.vector.tensor_tensor(
                        losers[:], boosted[:], w[:], mybir.AluOpType.subtract
                    )
                    nc.sync.dma_start(out[m], losers[:])
    nc.compile()
    return nc


def _get_nc(k_active: int):
    nc = _BUILD_CACHE.get(k_active)
    if nc is None:
        nc = _BUILD_CACHE[k_active] = _build(k_active)
    return nc


def _bf16_split(x):
    """x (f32) -> (hi, lo) bf16 arrays with hi + lo ~ x (17-bit mantissa)."""
    hi = x.astype(ml_dtypes.bfloat16)
    lo = (x - hi.astype(np.float32)).astype(ml_dtypes.bfloat16)
    return hi, lo


def kernel(input_vector, connections, boosting_factors, num_active):
    x = np.ascontiguousarray(input_vector, dtype=np.float32).reshape(-1, D)

    b = np.ascontiguousarray(boosting_factors, dtype=np.float32)
    k = min(int(num_active), NCOL)
    n_tok = x.shape[0]
    assert n_tok == N_CORES * TOK_PER_CORE, n_tok

    nc = _get_nc(k)

    # x^T laid out as [core, m, ks(part), kc*128 + t]
    xt = np.ascontiguousarray(x.T)                       # [D, n_tok]
    xt = xt.reshape(KC, 128, N_CORES, M_TILES, 128)      # [kc, ks, core, m, t]
    xt = xt.transpose(2, 3, 1, 0, 4)                     # [core, m, ks, kc, t]
    xt = np.ascontiguousarray(xt).reshape(N_CORES, M_TILES, 128, KC * 128)
    xt_hi, xt_lo = _bf16_split(xt)

    # C^T laid out as [ks(part), kc*NCOL + col]; exact in bf16
    ct = np.ascontiguousarray(connections.T, dtype=np.float32)  # [D, NCOL]
    ct = ct.reshape(KC, 128, NCOL).transpose(1, 0, 2)
    ct = np.ascontiguousarray(ct).reshape(128, KC * NCOL).astype(ml_dtypes.bfloat16)

    bcast = np.broadcast_to(b, (128, NCOL))
    bcast = np.ascontiguousarray(bcast)

    in_maps = [
        {"xhi": xt_hi[cidx], "xlo": xt_lo[cidx], "ct": ct, "bc": bcast}
        for cidx in range(N_CORES)
    ]
    res = run_bass_kernel_spmd(nc, in_maps, core_ids=list(range(N_CORES)))
    outs = [r["out"].reshape(TOK_PER_CORE, NCOL) for r in res.results]
    full = np.concatenate(outs, axis=0)
    return full.reshape(input_vector.shape[0], input_vector.shape[1], NCOL)



# revision 2
# speedup vs baseline: 1.6135x; 1.6135x over previous
"""HTM spatial-pooler kernel for Trainium2 (8 NeuronCores, data-parallel over tokens).

Computes, for x = input_vector reshaped to [4096 tokens, 4096]:
    overlap = x @ C^T               (C = connections [2048, 4096], binary)
    boosted = overlap * boost       (per-column boosting factors)
    masked  = where(boosted >= kth_largest_per_row(boosted, k), boosted, 0)

Strategy per core (512 tokens):
  - Single fp16 matmul pass (C is exact in fp16; x rounds to fp16, which
    perturbs boosted by ~7e-3 rms — small enough that only ~24 of the 164k
    active entries flip across the top-k threshold, rel-err ~1.2e-2).
    fp32 PSUM accumulation; C^T stays resident in SBUF (16 MB fp16).
  - Tokens on PSUM partitions, columns on the free axis, so the per-row top-k
    runs on the DVE with max8/match_replace; the k-th value is used as a
    threshold and the mask applied with a fused scalar_tensor_tensor
    (boosted >= thr) * boosted (matches the reference's tie semantics).
  - Output streamed out as fp16 (values < 2048, so ulp <= 1; upcast on host).
"""
import math

import numpy as np
import ml_dtypes

import concourse.bacc as bacc
import concourse.mybir as mybir
from concourse import tile
from concourse.bass_utils import run_bass_kernel_spmd

F16 = mybir.dt.float16
F32 = mybir.dt.float32

N_CORES = 8
TOK_PER_CORE = 512
M_TILES = 4          # 128-token tiles per core
D = 4096             # input size (contraction)
KC = D // 128        # 32 contraction chunks
NCOL = 2048          # minicolumns
NCH = NCOL // 512    # 4 psum column chunks

_BUILD_CACHE = {}


def _build(k_active: int):
    nc = bacc.Bacc("TRN2", target_bir_lowering=False)
    xq = nc.dram_tensor("xq", [M_TILES, 128, KC * 128], F16, kind="ExternalInput")
    ct = nc.dram_tensor("ct", [128, KC * NCOL], F16, kind="ExternalInput")
    bc = nc.dram_tensor("bc", [128, NCOL], F32, kind="ExternalInput")
    out = nc.dram_tensor("out", [M_TILES, 128, NCOL], F16, kind="ExternalOutput")

    rounds = max(1, math.ceil(k_active / 8))
    t_idx = (k_active - 1) % 8

    with tile.TileContext(nc) as tc:
        with (
            tc.tile_pool(name="cpool", bufs=1) as cpool,
            tc.tile_pool(name="xpool", bufs=2) as xpool,
            tc.tile_pool(name="psum", bufs=2, space="PSUM") as pspool,
            tc.tile_pool(name="work", bufs=1) as wpool,
            tc.tile_pool(name="bpool", bufs=2) as bpool,
            tc.tile_pool(name="lpool", bufs=2) as lpool,
        ):
            XCH = 4                      # x loaded in 4 kc-block chunks
            KCB = KC // XCH              # 8 kc per chunk

            def load_x(m):
                chunks = []
                for j in range(XCH):
                    xj = xpool.tile([128, KCB * 128], F16, tag=f"x{j}")
                    nc.sync.dma_start(
                        xj[:], xq[m][:, j * KCB * 128:(j + 1) * KCB * 128])
                    chunks.append(xj)
                return chunks

            # C^T resident as per-kc chunk tiles so the first matmuls only
            # gate on the first chunk's DMA, not the full 16 MB load.
            ct_tiles = []

            def load_ct(kc):
                t = cpool.tile([128, NCOL], F16, tag=f"ct{kc}")
                nc.sync.dma_start(t[:], ct[:, kc * NCOL:(kc + 1) * NCOL])
                ct_tiles.append(t)

            load_ct(0)
            load_ct(1)
            next_xchunks = load_x(0)
            for kc in range(2, KC):
                load_ct(kc)
            bc_t = cpool.tile([128, NCOL], F32)
            nc.sync.dma_start(bc_t[:], bc[:])

            for m in range(M_TILES):
                xchunks = next_xchunks
                if m + 1 < M_TILES:
                    next_xchunks = load_x(m + 1)

                ps = pspool.tile([128, NCOL], F32)
                for kc in range(KC):
                    lhsT = xchunks[kc // KCB][:, (kc % KCB) * 128:
                                              (kc % KCB) * 128 + 128]
                    for n in range(NCH):
                        nc.tensor.matmul(
                            ps[:, n * 512:(n + 1) * 512],
                            lhsT,
                            ct_tiles[kc][:, n * 512:(n + 1) * 512],
                            start=(kc == 0),
                            stop=(kc == KC - 1),
                        )

                boosted = bpool.tile([128, NCOL], F32, tag="boosted")
                nc.vector.tensor_tensor(
                    boosted[:], ps[:], bc_t[:], mybir.AluOpType.mult
                )

                # Segmented top-k: per-64-col-segment top-8 candidates
                # (a segment can contribute at most 8 to the top-k; for
                # k=40 the chance any segment holds >8 of the top-k is
                # ~2e-4 per row), then an exact k-th-largest on the 256
                # candidates, then threshold-mask the full row (same
                # `>= thr` tie semantics as the reference).
                SEG = 64
                NSEG = NCOL // SEG
                cands = wpool.tile([128, NSEG * 8], F32, tag="cands")
                for s in range(NSEG):
                    nc.vector.max(
                        cands[:, s * 8:(s + 1) * 8],
                        boosted[:, s * SEG:(s + 1) * SEG],
                    )
                tops = wpool.tile([128, 8 * rounds], F32, tag="tops")
                wc = wpool.tile([128, NSEG * 8], F32, tag="wc")
                src = cands
                for r in range(rounds):
                    m8 = tops[:, r * 8:(r + 1) * 8]
                    nc.vector.max(m8, src[:])
                    if r != rounds - 1:
                        nc.vector.match_replace(wc[:], m8, src[:], 0.0)
                        src = wc
                thr = tops[:, (rounds - 1) * 8 + t_idx:
                           (rounds - 1) * 8 + t_idx + 1]
                # masked = (boosted >= thr) * boosted, fused; emit fp16
                masked = lpool.tile([128, NCOL], F16, tag="masked")
                nc.vector.scalar_tensor_tensor(
                    masked[:], boosted[:], thr, boosted[:],
                    mybir.AluOpType.is_ge, mybir.AluOpType.mult,
                )
                nc.sync.dma_start(out[m], masked[:])
    nc.compile()
    return nc


def _get_nc(k_active: int):
    nc = _BUILD_CACHE.get(k_active)
    if nc is None:
        nc = _BUILD_CACHE[k_active] = _build(k_active)
    return nc


def kernel(input_vector, connections, boosting_factors, num_active):
    x = np.ascontiguousarray(input_vector, dtype=np.float32).reshape(-1, D)

    b = np.ascontiguousarray(boosting_factors, dtype=np.float32)
    k = min(int(num_active), NCOL)
    n_tok = x.shape[0]
    assert n_tok == N_CORES * TOK_PER_CORE, n_tok

    nc = _get_nc(k)

    # x^T laid out as [core, m, ks(part), kc*128 + t]
    xt = np.ascontiguousarray(x.T)                       # [D, n_tok]
    xt = xt.reshape(KC, 128, N_CORES, M_TILES, 128)      # [kc, ks, core, m, t]
    xt = xt.transpose(2, 3, 1, 0, 4)                     # [core, m, ks, kc, t]
    xt = np.ascontiguousarray(xt).reshape(N_CORES, M_TILES, 128, KC * 128)
    xt = xt.astype(np.float16)

    # C^T laid out as [ks(part), kc*NCOL + col]; exact in fp16
    ct = np.ascontiguousarray(connections.T, dtype=np.float32)  # [D, NCOL]
    ct = ct.reshape(KC, 128, NCOL).transpose(1, 0, 2)
    ct = np.ascontiguousarray(ct).reshape(128, KC * NCOL).astype(np.float16)

    bcast = np.broadcast_to(b, (128, NCOL))
    bcast = np.ascontiguousarray(bcast)

    in_maps = [
        {"xq": xt[cidx], "ct": ct, "bc": bcast}
        for cidx in range(N_CORES)
    ]
    res = run_bass_kernel_spmd(nc, in_maps, core_ids=list(range(N_CORES)))
    outs = [r["out"].astype(np.float32).reshape(TOK_PER_CORE, NCOL)
            for r in res.results]
    full = np.concatenate(outs, axis=0)
    return full.reshape(input_vector.shape[0], input_vector.shape[1], NCOL)


# revision 5
# speedup vs baseline: 2.2192x; 1.3754x over previous
"""HTM spatial-pooler kernel for Trainium2 (8 NeuronCores, data-parallel over tokens).

Computes, for x = input_vector reshaped to [4096 tokens, 4096]:
    overlap = x @ C^T               (C = connections [2048, 4096], binary)
    boosted = overlap * boost       (per-column boosting factors)
    masked  = where(boosted >= kth_largest_per_row(boosted, k), boosted, 0)

Strategy per core (512 tokens):
  - x is decomposed into three fp8-e4m3 "digits": x ~= d1 + d2/32 + d3/1024
    (residual < 2^-15 rms, i.e. effectively exact for the top-k). C is binary,
    exact in fp8. All matmuls run in fp8 DoubleRow perf mode (two contraction
    rows per PE pass), so the 3 digit passes cost 0.75x one bf16 pass.
  - Each digit pass accumulates into its own PSUM region at its own scale;
    they are combined during the boost multiply:
        boosted = (A + B/32 + C/1024) * boost
    (scales folded into scalar_tensor_tensor immediates / Act scale).
  - Work is ordered column-bank-major (4 banks of 512 columns): per
    (m-tile, bank) the 3 passes accumulate into 3 PSUM banks which drain
    while the next bank computes; 6 of 8 PSUM banks in flight.
  - C is loaded bank-major so the first m-tile is paced by the C stream
    without PSUM pressure.
  - Tokens sit on PSUM partitions, columns on the free axis; the per-row
    top-k threshold runs on the DVE (per-64-column max8 candidates, then
    iterated max8/match_replace on the 256 candidates), and the mask is a
    fused (boosted >= thr) * boosted scalar_tensor_tensor, emitted as fp16.
"""
import math

import numpy as np
import ml_dtypes

import concourse.bacc as bacc
import concourse.mybir as mybir
from concourse import tile
from concourse.bass_utils import run_bass_kernel_spmd

F8 = mybir.dt.float8e4
F16 = mybir.dt.float16
F32 = mybir.dt.float32
E4 = ml_dtypes.float8_e4m3

N_CORES = 8
TOK_PER_CORE = 512
M_TILES = 4          # 128-token tiles per core
D = 4096             # input size (contraction)
KC = D // 128        # 32 contraction chunks
KCP = KC // 2        # 16 DoubleRow chunk-pairs
NCOL = 2048          # minicolumns
NBANK = 4            # 512-column psum banks
BW = NCOL // NBANK   # 512

_BUILD_CACHE = {}


def _build(k_active: int):
    nc = bacc.Bacc("TRN2", target_bir_lowering=False)
    xd = [nc.dram_tensor(f"xd{p}", [M_TILES, 128, KC, 128], F8,
                         kind="ExternalInput") for p in range(3)]
    ct = nc.dram_tensor("ct", [NBANK, 128, KC, BW], F8, kind="ExternalInput")
    bc = nc.dram_tensor("bc", [128, NCOL], F32, kind="ExternalInput")
    out = nc.dram_tensor("out", [M_TILES, 128, NCOL], F16, kind="ExternalOutput")

    rounds = max(1, math.ceil(k_active / 8))
    t_idx = (k_active - 1) % 8
    DR = mybir.MatmulPerfMode.DoubleRow
    COPY = mybir.ActivationFunctionType.Copy

    with tile.TileContext(nc) as tc:
        with (
            tc.tile_pool(name="cpool", bufs=1) as cpool,
            tc.tile_pool(name="xpool", bufs=2) as xpool,
            tc.tile_pool(name="psum", bufs=2, space="PSUM") as pspool,
            tc.tile_pool(name="work", bufs=2) as wpool,
            tc.tile_pool(name="bpool", bufs=2) as bpool,
            tc.tile_pool(name="lpool", bufs=2) as lpool,
        ):
            # --- DMA program order ---------------------------------------
            # C bank-major in half-bank tiles (kc 0..15 / 16..31) so the
            # first matmuls gate on 1 MiB, interleaved with m0's digits.
            ctt = {}

            def load_ct(j, h):
                t = cpool.tile([128, KCP, BW], F8, tag=f"ct{j}{h}")
                nc.sync.dma_start(t[:], ct[j][:, h * KCP:(h + 1) * KCP, :])
                ctt[(j, h)] = t

            def load_x(m):
                tiles = []
                for p in range(3):
                    t = xpool.tile([128, KC, 128], F8, tag=f"x{p}")
                    nc.sync.dma_start(t[:], xd[p][m])
                    tiles.append(t)
                return tiles

            load_ct(0, 0)
            x_next = load_x(0)
            load_ct(0, 1)
            load_ct(1, 0)
            load_ct(1, 1)
            bc_t = cpool.tile([128, NCOL], F32)
            nc.sync.dma_start(bc_t[:], bc[:])
            for j in (2, 3):
                load_ct(j, 0)
                load_ct(j, 1)

            for m in range(M_TILES):
                xt = x_next
                if m + 1 < M_TILES:
                    x_next = load_x(m + 1)

                boosted = bpool.tile([128, NCOL], F32, tag="boosted")
                cands = wpool.tile([128, (NCOL // 64) * 8], F32, tag="cands")

                for j in range(NBANK):
                    psA = pspool.tile([128, BW], F32, tag="PA")
                    psB = pspool.tile([128, BW], F32, tag="PB")
                    psC = pspool.tile([128, BW], F32, tag="PC")
                    ps = [psA, psB, psC]
                    for p in range(3):
                        for i in range(KCP):
                            kl = 2 * i - (i // 8) * KCP  # kc within half-tile
                            nc.tensor.matmul(
                                ps[p][:],
                                xt[p][:, 2 * i:2 * i + 2, :],
                                ctt[(j, i // 8)][:, kl:kl + 2, :],
                                start=(i == 0),
                                stop=(i == KCP - 1),
                                perf_mode=DR,
                            )
                    # boosted_j = (A + B/32 + C/1024) * bc_j
                    tB = wpool.tile([128, BW], F32, tag="tB")
                    nc.scalar.activation(tB[:], ps[1][:], COPY, scale=1 / 32.0)
                    s2 = wpool.tile([128, BW], F32, tag="s2")
                    nc.vector.scalar_tensor_tensor(
                        s2[:], ps[2][:], 1 / 1024.0, tB[:],
                        mybir.AluOpType.mult, mybir.AluOpType.add)
                    s3 = wpool.tile([128, BW], F32, tag="s3")
                    nc.vector.scalar_tensor_tensor(
                        s3[:], ps[0][:], 1.0, s2[:],
                        mybir.AluOpType.mult, mybir.AluOpType.add)
                    bj = boosted[:, j * BW:(j + 1) * BW]
                    nc.gpsimd.tensor_tensor(
                        bj, s3[:], bc_t[:, j * BW:(j + 1) * BW],
                        mybir.AluOpType.mult)
                    if k_active <= 48:
                        # top-8 candidates per 64-column segment of this bank
                        for s in range(BW // 64):
                            sg = j * (BW // 64) + s
                            nc.vector.max(
                                cands[:, sg * 8:(sg + 1) * 8],
                                boosted[:, sg * 64:(sg + 1) * 64],
                            )

                if k_active <= 48:
                    NSEG = NCOL // 64
                    tops = wpool.tile([128, 8 * rounds], F32, tag="tops")
                    wc = wpool.tile([128, NSEG * 8], F32, tag="wc")
                    src = cands
                    for r in range(rounds):
                        m8 = tops[:, r * 8:(r + 1) * 8]
                        nc.vector.max(m8, src[:])
                        if r != rounds - 1:
                            nc.vector.match_replace(wc[:], m8, src[:], 0.0)
                            src = wc
                    thr = tops[:, (rounds - 1) * 8 + t_idx:
                               (rounds - 1) * 8 + t_idx + 1]
                    masked = lpool.tile([128, NCOL], F16, tag="masked")
                    nc.vector.scalar_tensor_tensor(
                        masked[:], boosted[:], thr, boosted[:],
                        mybir.AluOpType.is_ge, mybir.AluOpType.mult,
                    )
                    nc.sync.dma_start(out[m], masked[:])
                else:
                    # Exact full-width chain: zero the top-k in a working
                    # copy, then masked = boosted - working.
                    rem = k_active % 8
                    tops = wpool.tile([128, 8 * rounds], F32, tag="tops")
                    w = wpool.tile([128, NCOL], F32, tag="w")
                    src = boosted
                    for r in range(rounds):
                        m8 = tops[:, r * 8:(r + 1) * 8]
                        nc.vector.max(m8, src[:])
                        if r == rounds - 1 and rem:
                            nc.gpsimd.memset(m8[:, rem:], -1e30)
                        nc.vector.match_replace(w[:], m8, src[:], 0.0)
                        src = w
                    losers = lpool.tile([128, NCOL], F16, tag="losers")
                    nc.vector.tensor_tensor(
                        losers[:], boosted[:], w[:], mybir.AluOpType.subtract
                    )
                    nc.sync.dma_start(out[m], losers[:])
    nc.compile()
    return nc


def _get_nc(k_active: int):
    nc = _BUILD_CACHE.get(k_active)
    if nc is None:
        nc = _BUILD_CACHE[k_active] = _build(k_active)
    return nc


def kernel(input_vector, connections, boosting_factors, num_active):
    x = np.ascontiguousarray(input_vector, dtype=np.float32).reshape(-1, D)

    b = np.ascontiguousarray(boosting_factors, dtype=np.float32)
    k = min(int(num_active), NCOL)
    n_tok = x.shape[0]
    assert n_tok == N_CORES * TOK_PER_CORE, n_tok

    nc = _get_nc(k)

    # x^T laid out as [core, m, ks(part), kc, t]
    xt = np.ascontiguousarray(x.T)                       # [D, n_tok]
    xt = xt.reshape(KC, 128, N_CORES, M_TILES, 128)      # [kc, ks, core, m, t]
    xt = xt.transpose(2, 3, 1, 0, 4)                     # [core, m, ks, kc, t]
    xt = np.ascontiguousarray(xt)

    # fp8 digit decomposition: x ~= d1 + d2/32 + d3/1024
    d1 = xt.astype(E4)
    r1 = xt - d1.astype(np.float32)
    d2 = (r1 * 32.0).astype(E4)
    r2 = r1 - d2.astype(np.float32) / 32.0
    d3 = (r2 * 1024.0).astype(E4)

    # C^T bank-major: [bank, ks(part), kc, col-within-bank]; exact in fp8
    ct = np.ascontiguousarray(connections.T, dtype=np.float32)  # [D, NCOL]
    ct = ct.reshape(KC, 128, NBANK, BW).transpose(2, 1, 0, 3)
    ct = np.ascontiguousarray(ct).astype(E4)

    bcast = np.ascontiguousarray(np.broadcast_to(b, (128, NCOL)))

    in_maps = [
        {"xd0": d1[cidx], "xd1": d2[cidx], "xd2": d3[cidx],
         "ct": ct, "bc": bcast}
        for cidx in range(N_CORES)
    ]
    res = run_bass_kernel_spmd(nc, in_maps, core_ids=list(range(N_CORES)))
    outs = [r["out"].astype(np.float32).reshape(TOK_PER_CORE, NCOL)
            for r in res.results]
    full = np.concatenate(outs, axis=0)
    return full.reshape(input_vector.shape[0], input_vector.shape[1], NCOL)


# revision 8
# speedup vs baseline: 2.2604x; 1.0186x over previous
"""HTM spatial-pooler kernel for Trainium2 (8 NeuronCores, data-parallel over tokens).

Computes, for x = input_vector reshaped to [4096 tokens, 4096]:
    overlap = x @ C^T               (C = connections [2048, 4096], binary)
    boosted = overlap * boost       (per-column boosting factors)
    masked  = where(boosted >= kth_largest_per_row(boosted, k), boosted, 0)

Strategy per core (512 tokens):
  - x is decomposed into three fp8-e4m3 "digits": x ~= d1 + d2/32 + d3/1024
    (residual < 1e-5 rms, i.e. effectively exact for the top-k). C is binary,
    exact in fp8. All matmuls run in fp8 DoubleRow perf mode (two contraction
    rows per PE pass), so the 3 digit passes cost 0.75x one bf16 pass.
  - Each digit pass accumulates into its own PSUM region at its own scale;
    they are combined during the boost multiply:
        boosted = (A + B/32 + C/1024) * boost
    (scales via Act copy-with-scale and scalar_tensor_tensor immediates).
  - Work is ordered column-bank-major (4 banks of 512 columns): per
    (m-tile, bank) the 3 passes accumulate into 3 PSUM banks which drain
    while the next bank computes; 6 of 8 PSUM banks in flight. Within a
    bank the contraction runs in quarters interleaved across the 3 digit
    passes so the first matmuls gate on a quarter of C, not all of it.
  - C is loaded bank-major (quarter tiles) so the first m-tile is paced by
    the C stream without PSUM pressure.
  - Tokens sit on PSUM partitions, columns on the free axis; the per-row
    top-k threshold runs on the DVE: per-64-column max8 candidates, a
    pre-reduction of banks 0-2's 192 candidates to their top-40 (hidden
    under bank 3's matmuls), then final rounds over just 104 values. The
    mask is a fused (boosted >= thr) * boosted scalar_tensor_tensor done in
    halves so the fp16 output DMA overlaps the second half.
"""
import math

import numpy as np
import ml_dtypes

import concourse.bacc as bacc
import concourse.mybir as mybir
from concourse import tile
from concourse.bass_utils import run_bass_kernel_spmd

F8 = mybir.dt.float8e4
F16 = mybir.dt.float16
F32 = mybir.dt.float32
E4 = ml_dtypes.float8_e4m3

N_CORES = 8
TOK_PER_CORE = 512
M_TILES = 4          # 128-token tiles per core
D = 4096             # input size (contraction)
KC = D // 128        # 32 contraction chunks
KCP = KC // 2        # 16 DoubleRow chunk-pairs
NCOL = 2048          # minicolumns
NBANK = 4            # 512-column psum banks
BW = NCOL // NBANK   # 512

_BUILD_CACHE = {}


def _build(k_active: int):
    nc = bacc.Bacc("TRN2", target_bir_lowering=False)
    xd = [nc.dram_tensor(f"xd{p}", [M_TILES, 128, KC, 128], F8,
                         kind="ExternalInput") for p in range(3)]
    ct = nc.dram_tensor("ct", [NBANK, 128, KC, BW], F8, kind="ExternalInput")
    bc = nc.dram_tensor("bc", [128, NCOL], F32, kind="ExternalInput")
    out = nc.dram_tensor("out", [M_TILES, 128, NCOL], F16, kind="ExternalOutput")

    rounds = max(1, math.ceil(k_active / 8))
    t_idx = (k_active - 1) % 8
    DR = mybir.MatmulPerfMode.DoubleRow
    COPY = mybir.ActivationFunctionType.Copy

    with tile.TileContext(nc) as tc:
        with (
            tc.tile_pool(name="cpool", bufs=1) as cpool,
            tc.tile_pool(name="xpool", bufs=2) as xpool,
            tc.tile_pool(name="psum", bufs=2, space="PSUM") as pspool,
            tc.tile_pool(name="work", bufs=2) as wpool,
            tc.tile_pool(name="bpool", bufs=2) as bpool,
            tc.tile_pool(name="lpool", bufs=2) as lpool,
        ):
            # --- DMA program order ---------------------------------------
            # C bank-major in quarter tiles (8 kc each) interleaved with m0's
            # x half-tiles and per-bank boost chunks, so the first matmuls
            # gate on ~768 KiB and the C stream paces m0 smoothly.
            ctq = {}

            def load_ct(j, q):
                t = cpool.tile([128, KC // 4, BW], F8, tag=f"ct{j}{q}")
                nc.sync.dma_start(t[:], ct[j][:, q * (KC // 4):(q + 1) * (KC // 4), :])
                ctq[(j, q)] = t

            def load_x_half(m, h, tiles=None):
                tiles = tiles or [None, None, None]
                for p in range(3):
                    t = xpool.tile([128, KC // 2, 128], F8, tag=f"x{p}h{h}")
                    nc.sync.dma_start(t[:], xd[p][m][:, h * (KC // 2):(h + 1) * (KC // 2), :])
                    tiles[p] = t
                return tiles

            bc_t = []

            def load_bc(j):
                t = cpool.tile([128, BW], F32, tag=f"bc{j}")
                nc.sync.dma_start(t[:], bc[:, j * BW:(j + 1) * BW])
                bc_t.append(t)

            load_ct(0, 0)
            xh0 = load_x_half(0, 0)
            load_ct(0, 1)
            xh1 = load_x_half(0, 1)
            load_ct(0, 2)
            load_bc(0)
            load_ct(0, 3)
            load_ct(1, 0)
            load_bc(1)
            load_ct(1, 1)
            load_ct(1, 2)
            load_bc(2)
            load_ct(1, 3)
            load_ct(2, 0)
            load_bc(3)
            for jq in ((2, 1), (2, 2), (2, 3), (3, 0), (3, 1), (3, 2), (3, 3)):
                load_ct(*jq)
            x_next = (xh0, xh1)

            NSEG_B = BW // 64            # 8 segments per bank
            PRE = 3 * NSEG_B * 8         # 192 candidates from banks 0-2
            use_seg = k_active <= 48

            for m in range(M_TILES):
                xh = x_next
                if m + 1 < M_TILES:
                    xa = load_x_half(m + 1, 0)
                    xb = load_x_half(m + 1, 1)
                    x_next = (xa, xb)

                boosted = bpool.tile([128, NCOL], F32, tag="boosted")
                # cands layout: [0:40] pre-reduced top-40 of banks 0-2,
                # [40:104] bank-3 segments, [104:296] banks 0-2 segments.
                cands = wpool.tile([128, 40 + 8 * NSEG_B + PRE], F32,
                                   tag="cands")
                pwc = wpool.tile([128, PRE], F32, tag="pwc")

                for j in range(NBANK):
                    psA = pspool.tile([128, BW], F32, tag="PA")
                    psB = pspool.tile([128, BW], F32, tag="PB")
                    psC = pspool.tile([128, BW], F32, tag="PC")
                    ps = [psA, psB, psC]
                    for q in range(4):
                        xq = xh[q // 2]
                        for p in range(3):
                            for i in range(4):
                                kl = (q % 2) * 8 + 2 * i
                                nc.tensor.matmul(
                                    ps[p][:],
                                    xq[p][:, kl:kl + 2, :],
                                    ctq[(j, q)][:, 2 * i:2 * i + 2, :],
                                    start=(q == 0 and i == 0),
                                    stop=(q == 3 and i == 3),
                                    perf_mode=DR,
                                )
                    # boosted_j = (A + B/32 + C/1024) * bc_j
                    tB = wpool.tile([128, BW], F32, tag="tB")
                    nc.scalar.activation(tB[:], psB[:], COPY, scale=1 / 32.0)
                    s2 = wpool.tile([128, BW], F32, tag="s2")
                    nc.vector.scalar_tensor_tensor(
                        s2[:], psC[:], 1 / 1024.0, tB[:],
                        mybir.AluOpType.mult, mybir.AluOpType.add)
                    s3 = wpool.tile([128, BW], F32, tag="s3")
                    nc.vector.scalar_tensor_tensor(
                        s3[:], psA[:], 1.0, s2[:],
                        mybir.AluOpType.mult, mybir.AluOpType.add)
                    bj = boosted[:, j * BW:(j + 1) * BW]
                    nc.gpsimd.tensor_tensor(
                        bj, s3[:], bc_t[j][:], mybir.AluOpType.mult)
                    if use_seg:
                        # top-8 candidates per 64-column segment of this bank
                        for s in range(NSEG_B):
                            off = (40 + s * 8) if j == 3 else \
                                104 + (j * NSEG_B + s) * 8
                            nc.vector.max(
                                cands[:, off:off + 8],
                                boosted[:, (j * NSEG_B + s) * 64:
                                        (j * NSEG_B + s) * 64 + 64],
                            )
                    if use_seg and j == 2:
                        # pre-reduce banks 0-2's 192 candidates to their
                        # top-40 while bank 3's matmuls run
                        src = cands[:, 104:104 + PRE]
                        for r in range(rounds):
                            m8 = cands[:, r * 8:(r + 1) * 8]
                            nc.vector.max(m8, src)
                            if r != rounds - 1:
                                nc.vector.match_replace(pwc[:], m8, src, 0.0)
                                src = pwc[:]

                if use_seg:
                    # exact k-th largest of top-40(banks 0-2) + bank-3 cands
                    NF = 40 + 8 * NSEG_B
                    tops = wpool.tile([128, 8 * rounds], F32, tag="tops")
                    wc = wpool.tile([128, NF], F32, tag="wc")
                    src = cands[:, 0:NF]
                    for r in range(rounds):
                        m8 = tops[:, r * 8:(r + 1) * 8]
                        nc.vector.max(m8, src)
                        if r != rounds - 1:
                            nc.vector.match_replace(wc[:], m8, src, 0.0)
                            src = wc[:]
                    thr = tops[:, (rounds - 1) * 8 + t_idx:
                               (rounds - 1) * 8 + t_idx + 1]
                    # masked = (boosted >= thr) * boosted, fused, in halves so
                    # the first fp16 output DMA overlaps the second half
                    masked = lpool.tile([128, NCOL], F16, tag="masked")
                    H = NCOL // 2
                    for hh in range(2):
                        nc.vector.scalar_tensor_tensor(
                            masked[:, hh * H:(hh + 1) * H],
                            boosted[:, hh * H:(hh + 1) * H], thr,
                            boosted[:, hh * H:(hh + 1) * H],
                            mybir.AluOpType.is_ge, mybir.AluOpType.mult,
                        )
                        nc.sync.dma_start(out[m][:, hh * H:(hh + 1) * H],
                                          masked[:, hh * H:(hh + 1) * H])
                else:
                    # Exact full-width chain: zero the top-k in a working
                    # copy, then masked = boosted - working.
                    rem = k_active % 8
                    tops = wpool.tile([128, 8 * rounds], F32, tag="tops")
                    w = wpool.tile([128, NCOL], F32, tag="w")
                    src = boosted
                    for r in range(rounds):
                        m8 = tops[:, r * 8:(r + 1) * 8]
                        nc.vector.max(m8, src[:])
                        if r == rounds - 1 and rem:
                            nc.gpsimd.memset(m8[:, rem:], -1e30)
                        nc.vector.match_replace(w[:], m8, src[:], 0.0)
                        src = w
                    losers = lpool.tile([128, NCOL], F16, tag="losers")
                    nc.vector.tensor_tensor(
                        losers[:], boosted[:], w[:], mybir.AluOpType.subtract
                    )
                    nc.sync.dma_start(out[m], losers[:])
    nc.compile()
    return nc


def _get_nc(k_active: int):
    nc = _BUILD_CACHE.get(k_active)
    if nc is None:
        nc = _BUILD_CACHE[k_active] = _build(k_active)
    return nc


def kernel(input_vector, connections, boosting_factors, num_active):
    x = np.ascontiguousarray(input_vector, dtype=np.float32).reshape(-1, D)

    b = np.ascontiguousarray(boosting_factors, dtype=np.float32)
    k = min(int(num_active), NCOL)
    n_tok = x.shape[0]
    assert n_tok == N_CORES * TOK_PER_CORE, n_tok

    nc = _get_nc(k)

    # x^T laid out as [core, m, ks(part), kc, t]
    xt = np.ascontiguousarray(x.T)                       # [D, n_tok]
    xt = xt.reshape(KC, 128, N_CORES, M_TILES, 128)      # [kc, ks, core, m, t]
    xt = xt.transpose(2, 3, 1, 0, 4)                     # [core, m, ks, kc, t]
    xt = np.ascontiguousarray(xt)

    # fp8 digit decomposition: x ~= d1 + d2/32 + d3/1024
    d1 = xt.astype(E4)
    r1 = xt - d1.astype(np.float32)
    d2 = (r1 * 32.0).astype(E4)
    r2 = r1 - d2.astype(np.float32) / 32.0
    d3 = (r2 * 1024.0).astype(E4)

    # C^T bank-major: [bank, ks(part), kc, col-within-bank]; exact in fp8
    ct = np.ascontiguousarray(connections.T, dtype=np.float32)  # [D, NCOL]
    ct = ct.reshape(KC, 128, NBANK, BW).transpose(2, 1, 0, 3)
    ct = np.ascontiguousarray(ct).astype(E4)

    bcast = np.ascontiguousarray(np.broadcast_to(b, (128, NCOL)))

    in_maps = [
        {"xd0": d1[cidx], "xd1": d2[cidx], "xd2": d3[cidx],
         "ct": ct, "bc": bcast}
        for cidx in range(N_CORES)
    ]
    res = run_bass_kernel_spmd(nc, in_maps, core_ids=list(range(N_CORES)))
    outs = [r["out"].astype(np.float32).reshape(TOK_PER_CORE, NCOL)
            for r in res.results]
    full = np.concatenate(outs, axis=0)
    return full.reshape(input_vector.shape[0], input_vector.shape[1], NCOL)
